# revision 1
# baseline (speedup 1.0000x reference)
"""Trainium2 Bass kernel for LBLHighwayBiLm.

Reference computation (per layer l of L=2, on [B=32, S=512, H=512] input):
  fwd/bwd depthwise window conv (5 taps, scalar weight per tap) with learned
  boundary pads, then NHW=2 highway layers per direction:
      proj = x @ W^T + b;  nl = relu(proj[:H]);  g = sigmoid(proj[H:])
      x = g * x + (1 - g) * nl
  output[l] = concat([f_out, b_out], -1)

Strategy: data-parallel over batch (4 per core x 8 cores). Feature-major
activations [h(part), hb, b, s] held in padded state buffers (width S+W,
pads at the left for the fwd chain, right for bwd) so each conv tap is a
full-width N=512 matmul on a shifted window, accumulated in PSUM with a
scaled-identity stationary operand. Highway matmuls contract h on
partitions with host-pre-transposed weights; all matmul operands are
float32r (fp22, 1 cycle/row; requires even free counts / 8B-aligned dst,
which full-width taps satisfy). Biases + relu/sigmoid are fused into the
scalar-engine PSUM evacuation. The highway combine is 3 vector-engine ops
ping-ponging between the two state buffers per direction so combines never
wait on matmuls that read their output buffer. The tensor engine is the
bottleneck (~99% busy), so ~1.3 of the 4 conv h-blocks run on the vector
engine as multiply-accumulate chains instead (DVE_SPLIT), balancing PE
against DVE; cost-model timeline: ~319 us/core.
"""

import numpy as np

B, S, H, L, W, NHW = 32, 512, 512, 2, 4, 2
NCORES = 8
BL = B // NCORES          # batch per core
P = 128
HB = H // P               # h blocks (4)
MB = 2 * H // P           # proj out blocks (8)
SW = S + W                # padded row width

_CACHE = {}


def _build_nc():
    import concourse.bass as bass
    import concourse.tile as tile
    from concourse import bacc, mybir

    f32 = mybir.dt.float32
    f32r = mybir.dt.float32r
    AF = mybir.ActivationFunctionType

    nc = bacc.Bacc("TRN2", target_bir_lowering=False)

    xt = nc.dram_tensor("xt", [BL, H, S], f32r, kind="ExternalInput")
    wt = nc.dram_tensor("wt", [L, 2, NHW, H, 2 * H], f32r, kind="ExternalInput")
    ids = nc.dram_tensor("ids", [L, 2, W + 1, P, P], f32r, kind="ExternalInput")
    padl = nc.dram_tensor("padl", [L, H, W], f32r, kind="ExternalInput")
    padr = nc.dram_tensor("padr", [L, H, W], f32r, kind="ExternalInput")
    hwb = nc.dram_tensor("hwb", [L, 2, NHW, P, MB], f32, kind="ExternalInput")
    ws = nc.dram_tensor("ws", [L, 2, W + 1], f32, kind="ExternalInput")
    out = nc.dram_tensor("out", [L, BL, 2 * H, S], f32, kind="ExternalOutput")

    with tile.TileContext(nc) as tc:
        with (
            tc.tile_pool(name="state", bufs=1) as state_pool,
            tc.tile_pool(name="singles", bufs=1) as singles,
            tc.tile_pool(name="ids", bufs=2) as ids_pool,
            tc.tile_pool(name="wt", bufs=2) as wt_pool,
            tc.tile_pool(name="evac", bufs=2) as evac_pool,
            tc.tile_pool(name="ps", bufs=2, space="PSUM") as ps_pool,
        ):
            hwb_sb = singles.tile([P, L, 2, NHW, MB], f32, tag="hwb", name="hwb_sb")
            nc.sync.dma_start(
                out=hwb_sb, in_=hwb.rearrange("l d i p m -> p l d i m")
            )
            ws_sb = singles.tile([P, L, 2, W + 1], f32, tag="ws", name="ws_sb")
            wsap = ws[:]
            nc.sync.dma_start(
                out=ws_sb,
                in_=bass.AP(tensor=wsap.tensor, offset=wsap.offset,
                            ap=[[0, P]] + list(wsap.ap)),
            )

            # ---- state buffers: two per direction, ping-pong across stages.
            # fwd chain: pads in cols [0, W), payload in [W, S+W)
            # bwd chain: payload in [0, S), pads in [S, S+W)
            bufs = {
                0: [state_pool.tile([P, HB, BL, SW], f32r, tag="fA", name="fA"),
                    state_pool.tile([P, HB, BL, SW], f32r, tag="fB", name="fB")],
                1: [state_pool.tile([P, HB, BL, SW], f32r, tag="bA", name="bA"),
                    state_pool.tile([P, HB, BL, SW], f32r, tag="bB", name="bB")],
            }
            OFF = {0: W, 1: 0}       # payload offset per direction
            PADOFF = {0: 0, 1: S}    # pad-slot offset per direction

            # load x per h-block so the first conv matmuls can start after
            # 1/4 of the transfer
            for hb in range(HB):
                xv = xt[:, hb * P:(hb + 1) * P, :].rearrange("b p s -> p b s")
                for d in range(2):
                    o = OFF[d]
                    nc.sync.dma_start(
                        out=bufs[d][0][:, hb, :, o:o + S], in_=xv
                    )

            # conv work split: per h-block, which batch rows go to DVE
            # (multiply-accumulate) vs PE (scaled-identity matmuls)
            DVE_SPLIT = {0: 0, 1: 0, 2: 1, 3: BL}  # hb -> first DVE b row

            def conv(l, d, ids_sb, src, dst):
                # payload(dst)[h, hb, b, t] = sum_k w[k] * src[h, hb, b, t+k]
                # (src padded: window [k, k+S) spans pads + payload correctly
                # for both directions). PE-saturated kernel: part of the work
                # runs on DVE as multiply-accumulate instead.
                o = OFF[d]
                for hb in range(HB):
                    nb = BL - DVE_SPLIT[hb]   # rows on DVE: [nb, BL)
                    # per-row chains so downstream matmuls unblock row by row
                    for b in range(nb, BL):
                        acc = dst[:, hb, b, o:o + S]
                        acc32 = acc.bitcast(f32)
                        nc.vector.tensor_scalar_mul(
                            acc, src[:, hb, b, 0:S].bitcast(f32),
                            ws_sb[:, l, d, 0:1],
                        )
                        for k in range(1, W + 1):
                            nc.vector.scalar_tensor_tensor(
                                acc,
                                src[:, hb, b, k:k + S].bitcast(f32),
                                ws_sb[:, l, d, k:k + 1],
                                acc32,
                                op0=mybir.AluOpType.mult,
                                op1=mybir.AluOpType.add,
                            )
                    if nb == 0:
                        continue
                    ps = ps_pool.tile([P, BL, S], f32, tag="ps", name="ps")
                    for k in range(W + 1):
                        for b in range(nb):
                            nc.tensor.matmul(
                                ps[:, b, :],
                                lhsT=ids_sb[:, k, :],
                                rhs=src[:, hb, b, k:k + S],
                                start=(k == 0),
                                stop=(k == W),
                            )
                    nc.scalar.copy(
                        out=dst[:, hb, :nb, o:o + S], in_=ps[:, :nb, :]
                    )

            def hw_linear(l, d, i, xin, xout):
                # payload(xout) = g * payload(xin) + (1-g) * relu(...)
                o = OFF[d]
                wt_sb = wt_pool.tile([P, HB, 2 * H], f32r, tag="wt", name="wt_sb")
                nc.sync.dma_start(
                    out=wt_sb,
                    in_=wt[l, d, i].rearrange("(kb p) o -> p kb o", p=P),
                )
                for j in range(HB):
                    nl = evac_pool.tile([P, BL, S], f32, tag="nl", name="nl")
                    g = evac_pool.tile([P, BL, S], f32, tag="g", name="g")
                    for half, (dst, fn) in enumerate(
                        ((nl, AF.Relu), (g, AF.Sigmoid))
                    ):
                        mb = j + HB * half
                        ps = ps_pool.tile([P, BL, S], f32, tag="ps", name="ps")
                        for kb in range(HB):
                            for b in range(BL):
                                nc.tensor.matmul(
                                    ps[:, b, :],
                                    lhsT=wt_sb[:, kb, mb * P:(mb + 1) * P],
                                    rhs=xin[:, kb, b, o:o + S],
                                    start=(kb == 0),
                                    stop=(kb == HB - 1),
                                )
                        nc.scalar.activation(
                            out=dst,
                            in_=ps[:],
                            func=fn,
                            bias=hwb_sb[:, l, d, i, mb:mb + 1],
                        )
                    # 3-op combine computed in place in xout's payload:
                    # xout = ((xin - nl) * g) + nl
                    xi32 = xin[:, j, :, o:o + S].bitcast(f32)
                    xoj = xout[:, j, :, o:o + S]
                    xo32 = xoj.bitcast(f32)
                    nc.vector.tensor_sub(xoj, xi32, nl)
                    nc.vector.tensor_mul(xoj, g, xo32)
                    nc.vector.tensor_add(xoj, xo32, nl)

            for l in range(L):
                ids_sb = {}
                for d in range(2):
                    ids_sb[d] = ids_pool.tile(
                        [P, W + 1, P], f32r, tag="ids", name="ids_sb"
                    )
                    nc.sync.dma_start(
                        out=ids_sb[d],
                        in_=ids[l, d].rearrange("k p m -> p k m"),
                    )
                    # layer pads into this layer's conv-input buffer
                    psrc = (padl if d == 0 else padr)[l].rearrange(
                        "(hb p) w -> p hb w", p=P
                    )
                    po = PADOFF[d]
                    for b in range(BL):
                        nc.sync.dma_start(
                            out=bufs[d][0][:, :, b, po:po + W], in_=psrc
                        )
                # conv both directions first: dense PE work while evacs run
                for d in range(2):
                    p, q = bufs[d]
                    conv(l, d, ids_sb[d], p, q)
                # interleave directions so one dir's combines overlap the
                # other dir's matmuls
                for i in range(NHW):
                    for d in range(2):
                        p, q = bufs[d]
                        if i == 0:
                            hw_linear(l, d, i, q, p)   # x = conv out (q) -> p
                        else:
                            hw_linear(l, d, i, p, q)   # final back into q
                for d in range(2):
                    p, q = bufs[d]
                    o = OFF[d]
                    hoff = 0 if d == 0 else H
                    # drain per (hb, b) so each DMA waits only on its own
                    # j-block combine, not the whole direction
                    for hb in range(HB):
                        for b in range(BL):
                            ov = out[l, b][hoff + hb * P:hoff + (hb + 1) * P, :]
                            nc.sync.dma_start(
                                out=ov,
                                in_=q[:, hb, b, o:o + S].bitcast(f32),
                            )
                    bufs[d] = [q, p]
    nc.finalize()
    return nc


def _get_nc():
    if "nc" not in _CACHE:
        _CACHE["nc"] = _build_nc()
    return _CACHE["nc"]


def _prep_shared(inputs):
    fwd_pads = np.asarray(inputs["fwd_pads"], np.float32)   # [L, W, H]
    bwd_pads = np.asarray(inputs["bwd_pads"], np.float32)
    fwd_ws = np.asarray(inputs["fwd_ws"], np.float32)       # [L, W+1]
    bwd_ws = np.asarray(inputs["bwd_ws"], np.float32)
    fwd_hw_W = np.asarray(inputs["fwd_hw_W"], np.float32)   # [L, NHW, 2H, H]
    fwd_hw_b = np.asarray(inputs["fwd_hw_b"], np.float32)   # [L, NHW, 2H]
    bwd_hw_W = np.asarray(inputs["bwd_hw_W"], np.float32)
    bwd_hw_b = np.asarray(inputs["bwd_hw_b"], np.float32)

    wt = np.empty((L, 2, NHW, H, 2 * H), np.float32)
    hwb = np.empty((L, 2, NHW, P, MB), np.float32)
    for l in range(L):
        for i in range(NHW):
            wt[l, 0, i] = fwd_hw_W[l, i].T
            wt[l, 1, i] = bwd_hw_W[l, i].T
            hwb[l, 0, i] = fwd_hw_b[l, i].reshape(MB, P).T
            hwb[l, 1, i] = bwd_hw_b[l, i].reshape(MB, P).T

    eye = np.eye(P, dtype=np.float32)
    ids = np.empty((L, 2, W + 1, P, P), np.float32)
    for l in range(L):
        for k in range(W + 1):
            ids[l, 0, k] = fwd_ws[l, k] * eye
            ids[l, 1, k] = bwd_ws[l, k] * eye

    ws = np.stack([fwd_ws, bwd_ws], axis=1)              # [L, 2, W+1]

    return {
        "ws": np.ascontiguousarray(ws),
        "wt": np.ascontiguousarray(wt),
        "ids": np.ascontiguousarray(ids),
        "padl": np.ascontiguousarray(fwd_pads.transpose(0, 2, 1)),  # [L, H, W]
        "padr": np.ascontiguousarray(bwd_pads.transpose(0, 2, 1)),
        "hwb": np.ascontiguousarray(hwb),
    }


def kernel(**inputs) -> np.ndarray:
    from concourse.bass_utils import run_bass_kernel_spmd

    x = np.asarray(inputs["inputs"], np.float32)            # [B, S, H]
    xt = np.ascontiguousarray(x.transpose(0, 2, 1))         # [B, H, S]
    shared = _prep_shared(inputs)

    nc = _get_nc()
    in_maps = []
    for c in range(NCORES):
        m = dict(shared)
        m["xt"] = np.ascontiguousarray(xt[c * BL:(c + 1) * BL])
        in_maps.append(m)
    res = run_bass_kernel_spmd(nc, in_maps, core_ids=list(range(NCORES)))
    _CACHE["last_res"] = res
    outs = [r["out"] for r in res.results]                  # [L, BL, 2H, S]
    full = np.concatenate(outs, axis=1)                     # [L, B, 2H, S]
    return np.ascontiguousarray(full.transpose(0, 1, 3, 2))  # [L, B, S, 2H]



# revision 4
# speedup vs baseline: 1.0148x; 1.0148x over previous
"""Trainium2 Bass kernel for LBLHighwayBiLm.

Reference computation (per layer l of L=2, on [B=32, S=512, H=512] input):
  fwd/bwd depthwise window conv (5 taps, scalar weight per tap) with learned
  boundary pads, then NHW=2 highway layers per direction:
      proj = x @ W^T + b;  nl = relu(proj[:H]);  g = sigmoid(proj[H:])
      x = g * x + (1 - g) * nl
  output[l] = concat([f_out, b_out], -1)

Strategy: data-parallel over batch (4 per core x 8 cores). Feature-major
fp16 activations [h(part), hb, b, s] in padded state buffers (width S+W,
pads left for the fwd chain, right for bwd). fp16 matmuls run at 1
cycle/row (same as fp32r) but halve DMA and unlock the DVE 2-byte fast
modes (tensor_scalar 4x, tensor_tensor 2x). PE does ONLY the highway
GEMMs (8 groups x 65536 cycles) plus the layer-0 fwd conv via
scaled-identity matmuls while it would otherwise idle at startup; all
other conv work runs on DVE as mul(4x)+add(2x) tap chains, and the
highway combines are split between DVE and the Pool/GPSIMD engine so no
single side engine exceeds the PE's ~218us of work. mb emission order is
interleaved (gate, nonlin, gate, ...) so each (nl_j, g_j) pair evacuates
early and combines + the next layer's conv stream on DVE during the
following matmul group, keeping PE back-to-back across layer boundaries.
"""

import numpy as np

B, S, H, L, W, NHW = 32, 512, 512, 2, 4, 2
NCORES = 8
BL = B // NCORES          # batch per core
P = 128
HB = H // P               # contraction blocks (4)
MBT = 2 * H // P          # proj out blocks (8)
SW = S + W                # padded row width
MB_ORDER = [4, 0, 5, 1, 6, 2, 7, 3]   # gate j / nonlin j pairs

_CACHE = {}


def _build_nc():
    import concourse.bass as bass
    import concourse.tile as tile
    from concourse import bacc, mybir

    f32 = mybir.dt.float32
    f16 = mybir.dt.float16
    AF = mybir.ActivationFunctionType

    nc = bacc.Bacc("TRN2", target_bir_lowering=False)

    xt = nc.dram_tensor("xt", [BL, H, S], f16, kind="ExternalInput")
    wt = nc.dram_tensor("wt", [L, 2, NHW, H, 2 * H], f16, kind="ExternalInput")
    ids = nc.dram_tensor("ids", [W + 1, P, P], f16, kind="ExternalInput")
    padl = nc.dram_tensor("padl", [L, H, W], f16, kind="ExternalInput")
    padr = nc.dram_tensor("padr", [L, H, W], f16, kind="ExternalInput")
    hwb = nc.dram_tensor("hwb", [L, 2, NHW, P, MBT], f32, kind="ExternalInput")
    ws = nc.dram_tensor("ws", [L, 2, W + 1], f32, kind="ExternalInput")
    out = nc.dram_tensor("out", [L, BL, 2 * H, S], f16, kind="ExternalOutput")

    with tile.TileContext(nc) as tc:
        with (
            tc.tile_pool(name="state", bufs=1) as state_pool,
            tc.tile_pool(name="singles", bufs=1) as singles,
            tc.tile_pool(name="nl", bufs=5) as nl_pool,
            tc.tile_pool(name="g", bufs=5) as g_pool,
            tc.tile_pool(name="tmp", bufs=2) as tmp_pool,
            tc.tile_pool(name="ps", bufs=2, space="PSUM") as ps_pool,
        ):
            hwb_sb = singles.tile([P, L, 2, NHW, MBT], f32, tag="hwb", name="hwb_sb")
            nc.sync.dma_start(out=hwb_sb, in_=hwb.rearrange("l d i p m -> p l d i m"))
            ws_sb = singles.tile([P, L, 2, W + 1], f32, tag="ws", name="ws_sb")
            wsap = ws[:]
            nc.sync.dma_start(
                out=ws_sb,
                in_=bass.AP(tensor=wsap.tensor, offset=wsap.offset,
                            ap=[[0, P]] + list(wsap.ap)),
            )
            ids_sb = singles.tile([P, W + 1, P], f16, tag="ids", name="ids_sb")
            nc.sync.dma_start(out=ids_sb, in_=ids.rearrange("k p m -> p k m"))

            # ---- state buffers. fwd chain: pads cols [0, W), payload [W, S+W)
            # bwd chain: payload [0, S), pads [S, S+W). Per layer l the conv
            # source is bufs[d][l % 2], dest bufs[d][1 - l % 2]; i0 writes back
            # into the source, i1 back into the dest (which is DMAed out).
            bufs = {
                0: [state_pool.tile([P, HB, BL, SW], f16, tag="fA", name="fA"),
                    state_pool.tile([P, HB, BL, SW], f16, tag="fB", name="fB")],
                1: [state_pool.tile([P, HB, BL, SW], f16, tag="bA", name="bA"),
                    state_pool.tile([P, HB, BL, SW], f16, tag="bB", name="bB")],
            }
            OFF = {0: W, 1: 0}       # payload offset per direction
            PADOFF = {0: 0, 1: S}    # pad-slot offset per direction

            wt_sb = singles.tile([P, L, 2, NHW, HB, 2 * H], f16, tag="wt",
                                 name="wt_sb")

            # x per h-block; first conv work starts after 1/4 of the transfer
            for hb in range(HB):
                xv = xt[:, hb * P:(hb + 1) * P, :].rearrange("b p s -> p b s")
                for d in range(2):
                    o = OFF[d]
                    nc.sync.dma_start(out=bufs[d][0][:, hb, :, o:o + S], in_=xv)
            # weights in group-consumption order
            for (l, d, i) in [(0, 0, 0), (0, 1, 0), (0, 0, 1), (0, 1, 1),
                              (1, 0, 0), (1, 1, 0), (1, 0, 1), (1, 1, 1)]:
                nc.sync.dma_start(
                    out=wt_sb[:, l, d, i],
                    in_=wt[l, d, i].rearrange("(kb p) o -> p kb o", p=P),
                )
            # pads for both layers up front (pad columns are never written by
            # compute, so no hazards): layer l pads -> that layer's conv source
            for l in range(L):
                for d in range(2):
                    psrc = (padl if d == 0 else padr)[l].rearrange(
                        "(hb p) w -> p hb w", p=P
                    )
                    po = PADOFF[d]
                    for b in range(BL):
                        nc.sync.dma_start(
                            out=bufs[d][l][:, :, b, po:po + W], in_=psrc
                        )

            def conv_dve_hb(l, d, hb):
                # payload(dst)[., hb, b, t] = sum_k w_k * src[., hb, b, t+k]
                src, dst = bufs[d][l % 2], bufs[d][1 - l % 2]
                o = OFF[d]
                acc = dst[:, hb, :, o:o + S]
                nc.vector.tensor_scalar_mul(
                    acc, src[:, hb, :, 0:S], ws_sb[:, l, d, 0:1]
                )
                for k in range(1, W + 1):
                    t = tmp_pool.tile([P, BL, S], f16, tag="tmp", name="tmp")
                    nc.vector.tensor_scalar_mul(
                        t, src[:, hb, :, k:k + S], ws_sb[:, l, d, k:k + 1]
                    )
                    nc.vector.tensor_add(acc, acc, t)

            def conv_pe_hb(l, d, hb):
                # scaled-identity matmul conv; ids holds fwd_ws[0] so only
                # valid for (l=0, d=0) — used while PE is idle at startup
                src, dst = bufs[d][l % 2], bufs[d][1 - l % 2]
                o = OFF[d]
                ps = ps_pool.tile([P, BL, S], f32, tag="ps", name="ps")
                for k in range(W + 1):
                    for b in range(BL):
                        nc.tensor.matmul(
                            ps[:, b, :], lhsT=ids_sb[:, k, :],
                            rhs=src[:, hb, b, k:k + S],
                            start=(k == 0), stop=(k == W),
                        )
                nc.scalar.copy(out=dst[:, hb, :, o:o + S], in_=ps[:])

            def mm_group(l, d, i):
                o = OFF[d]
                xin = bufs[d][1 - l % 2] if i == 0 else bufs[d][l % 2]
                nl_t, g_t = {}, {}
                for mb in MB_ORDER:
                    half, j = divmod(mb, HB)
                    ps = ps_pool.tile([P, BL, S], f32, tag="ps", name="ps")
                    for kb in range(HB):
                        for b in range(BL):
                            nc.tensor.matmul(
                                ps[:, b, :],
                                lhsT=wt_sb[:, l, d, i, kb, mb * P:(mb + 1) * P],
                                rhs=xin[:, kb, b, o:o + S],
                                start=(kb == 0), stop=(kb == HB - 1),
                            )
                    if half == 0:
                        dst = nl_pool.tile([P, BL, S], f16, tag="nl", name="nl")
                        nl_t[j], fn = dst, AF.Relu
                    else:
                        dst = g_pool.tile([P, BL, S], f16, tag="g", name="g")
                        g_t[j], fn = dst, AF.Sigmoid
                    nc.scalar.activation(
                        out=dst[:], in_=ps[:], func=fn,
                        bias=hwb_sb[:, l, d, i, mb:mb + 1],
                    )
                return nl_t, g_t

            def combine_j(eng, l, d, i, j, nl_t, g_t):
                # payload(xout) = g * payload(xin) + (1-g) * nl
                #               = nl + g * (xin - nl), 3 tensor_tensor ops
                o = OFF[d]
                xin = bufs[d][1 - l % 2] if i == 0 else bufs[d][l % 2]
                xout = bufs[d][l % 2] if i == 0 else bufs[d][1 - l % 2]
                x = xin[:, j, :, o:o + S]
                xo = xout[:, j, :, o:o + S]
                eng.tensor_sub(xo, x, nl_t[j][:])
                eng.tensor_mul(xo, xo, g_t[j][:])
                eng.tensor_add(xo, xo, nl_t[j][:])

            def out_dma(l, d, hb):
                q = bufs[d][1 - l % 2]
                o = OFF[d]
                hoff = 0 if d == 0 else H
                ov = out[l][:, hoff + hb * P:hoff + (hb + 1) * P, :].rearrange(
                    "b p s -> p b s"
                )
                nc.sync.dma_start(out=ov, in_=q[:, hb, :, o:o + S])

            # layer-0 convs: fwd on PE (otherwise idle), bwd on DVE
            for hb in range(HB):
                conv_pe_hb(0, 0, hb)
            for hb in range(HB):
                conv_dve_hb(0, 1, hb)

            for l in range(L):
                for i in range(NHW):
                    for d in range(2):
                        nl_t, g_t = mm_group(l, d, i)
                        if i == 0:
                            combine_j(nc.vector, l, d, i, 0, nl_t, g_t)
                            combine_j(nc.vector, l, d, i, 1, nl_t, g_t)
                            combine_j(nc.gpsimd, l, d, i, 2, nl_t, g_t)
                            combine_j(nc.gpsimd, l, d, i, 3, nl_t, g_t)
                        else:
                            for j in range(HB):
                                combine_j(nc.vector, l, d, i, j, nl_t, g_t)
                                if l + 1 < L:
                                    conv_dve_hb(l + 1, d, j)
                                out_dma(l, d, j)
    nc.finalize()
    return nc


def _get_nc():
    if "nc" not in _CACHE:
        _CACHE["nc"] = _build_nc()
    return _CACHE["nc"]


def _prep_shared(inputs):
    fwd_pads = np.asarray(inputs["fwd_pads"], np.float32)   # [L, W, H]
    bwd_pads = np.asarray(inputs["bwd_pads"], np.float32)
    fwd_ws = np.asarray(inputs["fwd_ws"], np.float32)       # [L, W+1]
    bwd_ws = np.asarray(inputs["bwd_ws"], np.float32)
    fwd_hw_W = np.asarray(inputs["fwd_hw_W"], np.float32)   # [L, NHW, 2H, H]
    fwd_hw_b = np.asarray(inputs["fwd_hw_b"], np.float32)   # [L, NHW, 2H]
    bwd_hw_W = np.asarray(inputs["bwd_hw_W"], np.float32)
    bwd_hw_b = np.asarray(inputs["bwd_hw_b"], np.float32)

    wt = np.empty((L, 2, NHW, H, 2 * H), np.float16)
    hwb = np.empty((L, 2, NHW, P, MBT), np.float32)
    for l in range(L):
        for i in range(NHW):
            wt[l, 0, i] = fwd_hw_W[l, i].T
            wt[l, 1, i] = bwd_hw_W[l, i].T
            hwb[l, 0, i] = fwd_hw_b[l, i].reshape(MBT, P).T
            hwb[l, 1, i] = bwd_hw_b[l, i].reshape(MBT, P).T

    eye = np.eye(P, dtype=np.float32)
    ids = np.empty((W + 1, P, P), np.float16)
    for k in range(W + 1):
        ids[k] = fwd_ws[0, k] * eye

    ws = np.stack([fwd_ws, bwd_ws], axis=1)              # [L, 2, W+1]

    return {
        "ws": np.ascontiguousarray(ws),
        "wt": np.ascontiguousarray(wt),
        "ids": np.ascontiguousarray(ids),
        "padl": np.ascontiguousarray(
            fwd_pads.transpose(0, 2, 1).astype(np.float16)),   # [L, H, W]
        "padr": np.ascontiguousarray(
            bwd_pads.transpose(0, 2, 1).astype(np.float16)),
        "hwb": np.ascontiguousarray(hwb),
    }


def kernel(**inputs) -> np.ndarray:
    from concourse.bass_utils import run_bass_kernel_spmd

    x = np.asarray(inputs["inputs"], np.float32)            # [B, S, H]
    xt = np.ascontiguousarray(
        x.transpose(0, 2, 1)).astype(np.float16)            # [B, H, S]
    shared = _prep_shared(inputs)

    nc = _get_nc()
    in_maps = []
    for c in range(NCORES):
        m = dict(shared)
        m["xt"] = np.ascontiguousarray(xt[c * BL:(c + 1) * BL])
        in_maps.append(m)
    res = run_bass_kernel_spmd(nc, in_maps, core_ids=list(range(NCORES)))
    _CACHE["last_res"] = res
    outs = [r["out"] for r in res.results]                  # [L, BL, 2H, S] f16
    full = np.concatenate(outs, axis=1)                     # [L, B, 2H, S]
    return np.ascontiguousarray(
        full.transpose(0, 1, 3, 2)).astype(np.float32)      # [L, B, S, 2H]
